# revision 14
# baseline (speedup 1.0000x reference)
"""Trainium2 Bass kernel for nn_DarkCLoss: loss = -mean(|maxpool3d_{3,35,35}(1-x)|).

Math: with p=35 and -inf padding, the reference is
    loss = -mean(1 - minpool2d_35x35(min_c x)) = mean(minpool) - 1
so we compute the 2D sliding-window min (window 35, stride 1, +inf pads)
of the channel-min, sum it, and finish on the host.

Sharding: pure data-parallel, 2 images per core across 8 cores; each core
returns its partial sum of the pooled map; host combines (the scalar
all-reduce from the sharding hint, done on host).

Device algorithm per image (all pooling exact in bf16; inputs shipped as
bf16 — the pooled term is ~2.7e-4 of the loss, so bf16 rounding of the
input perturbs the result by ~1e-6 relative):
  - rows are laid out h = 128*hc + p; the 4 row-blocks become +inf-padded
    548-wide segments side by side in the free dim.
  - work is split into half-image streams (2 segments each) so DMA,
    VectorE, ScalarE and PE pipelines of the two images interleave.
  - channel min: DVE tensor_tensor on FLAT [128, 1096] views (flat 2D
    APs keep the DVE in its 2x bf16 perf mode; segment-crossing reads
    only pollute positions no valid output depends on, because every
    valid 35-window's dependency cone stays inside one padded segment).
  - sliding-min-35 = log2 doubling chain of shifted flat tensor_tensor
    mins (shifts 1,2,4,8,16,3).  Odd shifts are made 4-byte aligned by
    materializing the shifted operand with a ScalarE copy, so every DVE
    op stays in 2x mode.
  - PE transposes [128,128] blocks into PSUM; ScalarE copies PSUM into
    the padded H buffer; same chain along H on transposed halves.
  - PE ones-matmul accumulates the partition sums of the pooled map into
    one PSUM bank across both images; one DVE reduce drains it to a
    scalar that is DMA'd out.
"""

import numpy as np
import ml_dtypes

import concourse.bacc as bacc
import concourse.tile as tile
import concourse.mybir as mybir
from concourse.alu_op_type import AluOpType
from concourse.bass_utils import run_bass_kernel_spmd
from concourse.masks import make_identity

N_CORES = 8
B, C, H, W = 16, 3, 512, 512
B_LOC = B // N_CORES          # images per core
K = 35                        # pool window
PAD_L = 18                    # left pad (data starts 4B-aligned)
SEG = 548                     # padded segment width (= 18 + 512 + 18)
HC = 4                        # 512 rows = 4 blocks of 128 partitions
HALF = 2 * SEG                # 1096: one half-image stream (2 segments)
INF = float("inf")

# chain op widths for a 2-segment stream: SEG + per-segment need
W_M2, W_D2, W_D4, W_D8, W_D16, W_FIN = 1094, 1092, 1088, 1080, 1064, 1062

_CACHE = {}


def _chain_half(nc, pool, buf2, base, tag):
    """Sliding-min-35 over two padded segments buf2[:, base:base+HALF].

    buf2: flat [128, >=base+HALF] bf16 AP with inf pads.  Returns a flat
    [128, HALF] tile whose columns SEG*s + (1..512), s in {0,1}, hold
    the valid window mins.  All DVE ops are flat 2D and 4B-aligned (odd
    shifts via ScalarE shadow copies) -> 2x bf16 mode.
    """
    bf16 = mybir.dt.bfloat16
    mn = AluOpType.min

    def tl(name):
        return pool.tile([128, HALF], bf16, name=name, tag=name, bufs=3)

    sh1 = tl(f"sh1{tag}")
    nc.scalar.copy(out=sh1[:, 0:W_M2], in_=buf2[:, base + 1:base + 1 + W_M2])
    m2 = tl(f"cha{tag}")
    nc.vector.tensor_tensor(
        out=m2[:, 0:W_M2], in0=buf2[:, base:base + W_M2],
        in1=sh1[:, 0:W_M2], op=mn)
    m4 = tl(f"chb{tag}")
    nc.vector.tensor_tensor(
        out=m4[:, 0:W_D2], in0=m2[:, 0:W_D2], in1=m2[:, 2:W_D2 + 2], op=mn)
    m8 = tl(f"chc{tag}")
    nc.vector.tensor_tensor(
        out=m8[:, 0:W_D4], in0=m4[:, 0:W_D4], in1=m4[:, 4:W_D4 + 4], op=mn)
    m16 = tl(f"chd{tag}")
    nc.vector.tensor_tensor(
        out=m16[:, 0:W_D8], in0=m8[:, 0:W_D8], in1=m8[:, 8:W_D8 + 8], op=mn)
    m32 = tl(f"che{tag}")
    nc.vector.tensor_tensor(
        out=m32[:, 0:W_D16], in0=m16[:, 0:W_D16], in1=m16[:, 16:W_D16 + 16],
        op=mn)
    sh3 = tl(f"sh3{tag}")
    nc.scalar.copy(out=sh3[:, 0:W_FIN], in_=m32[:, 3:3 + W_FIN])
    out = tl(f"out{tag}")
    nc.vector.tensor_tensor(
        out=out[:, 0:W_FIN], in0=m32[:, 0:W_FIN], in1=sh3[:, 0:W_FIN], op=mn)
    return out


FW_M2, FW_D2, FW_D4, FW_D8, FW_D16, FW_FIN = 2190, 2188, 2184, 2176, 2160, 2158


def _chain_full(nc, pool, buf2, tag):
    """Sliding-min-35 over four padded segments (flat [128, 4*SEG])."""
    bf16 = mybir.dt.bfloat16
    mn = AluOpType.min

    def tl(name):
        return pool.tile([128, 4 * SEG], bf16, name=name, tag=name, bufs=2)

    sh1 = tl(f"fsh1{tag}")
    nc.scalar.copy(out=sh1[:, 0:FW_M2], in_=buf2[:, 1:1 + FW_M2])
    m2 = tl(f"fcha{tag}")
    nc.vector.tensor_tensor(
        out=m2[:, 0:FW_M2], in0=buf2[:, 0:FW_M2], in1=sh1[:, 0:FW_M2], op=mn)
    m4 = tl(f"fchb{tag}")
    nc.vector.tensor_tensor(
        out=m4[:, 0:FW_D2], in0=m2[:, 0:FW_D2], in1=m2[:, 2:FW_D2 + 2], op=mn)
    m8 = tl(f"fchc{tag}")
    nc.vector.tensor_tensor(
        out=m8[:, 0:FW_D4], in0=m4[:, 0:FW_D4], in1=m4[:, 4:FW_D4 + 4], op=mn)
    m16 = tl(f"fchd{tag}")
    nc.vector.tensor_tensor(
        out=m16[:, 0:FW_D8], in0=m8[:, 0:FW_D8], in1=m8[:, 8:FW_D8 + 8], op=mn)
    m32 = tl(f"fche{tag}")
    nc.vector.tensor_tensor(
        out=m32[:, 0:FW_D16], in0=m16[:, 0:FW_D16], in1=m16[:, 16:FW_D16 + 16],
        op=mn)
    sh3 = tl(f"fsh3{tag}")
    nc.scalar.copy(out=sh3[:, 0:FW_FIN], in_=m32[:, 3:3 + FW_FIN])
    out = tl(f"fout{tag}")
    nc.vector.tensor_tensor(
        out=out[:, 0:FW_FIN], in0=m32[:, 0:FW_FIN], in1=sh3[:, 0:FW_FIN], op=mn)
    return out


def _build():
    if "nc" in _CACHE:
        return _CACHE["nc"]
    bf16 = mybir.dt.bfloat16
    f32 = mybir.dt.float32
    mn = AluOpType.min

    nc = bacc.Bacc("TRN2", target_bir_lowering=False, debug=False)
    x = nc.dram_tensor("x", [B_LOC, C, H, W], bf16, kind="ExternalInput")
    out_d = nc.dram_tensor("out", [1, 1], f32, kind="ExternalOutput")

    with tile.TileContext(nc) as tc:
        with (
            tc.tile_pool(name="consts", bufs=1) as consts,
            tc.tile_pool(name="work", bufs=2) as work,
            tc.tile_pool(name="pswork", bufs=2, space="PSUM") as pswork,
            tc.tile_pool(name="psacc", bufs=1, space="PSUM") as psacc,
        ):
            ident = consts.tile([128, 128], bf16)
            make_identity(nc, ident)
            ones = consts.tile([128, 1], bf16)
            nc.vector.memset(ones, 1.0)
            acc = psacc.tile([1, 512], f32)

            pts, hbufs = [], []
            for b in range(B_LOC):
                pt = pswork.tile([128, HC, 512], bf16, name="pt")
                hbuf = work.tile([128, HC, SEG], bf16, name="hbuf", tag="hbuf")
                wbuf = work.tile(
                    [128, HC, SEG], bf16, name="wbuf", tag="wbuf", bufs=2)
                wb2 = wbuf.rearrange("p a b -> p (a b)")
                for hw in range(2):          # half ch-stages fill wbuf
                    ct = []
                    for c in range(C):
                        t = work.tile(
                            [128, 2, SEG], bf16, name=f"c{c}", tag=f"c{c}",
                            bufs=3)
                        src = x[b, c, 256 * hw:256 * (hw + 1)].rearrange(
                            "(hc p) w -> p hc w", p=128)
                        eng = nc.sync if c % 2 == 0 else nc.scalar
                        eng.dma_start(out=t[:, :, PAD_L:PAD_L + 512], in_=src)
                        ct.append(t)
                    cf = [t.rearrange("p a b -> p (a b)") for t in ct]
                    t1 = work.tile(
                        [128, HALF], bf16, name="t1", tag="t1", bufs=3)
                    nc.vector.tensor_tensor(out=t1, in0=cf[0], in1=cf[1], op=mn)
                    nc.vector.tensor_tensor(
                        out=wb2[:, HALF * hw:HALF * (hw + 1)], in0=t1,
                        in1=cf[2], op=mn)
                nc.gpsimd.memset(wbuf[:, :, 0:PAD_L], INF)
                nc.gpsimd.memset(wbuf[:, :, PAD_L + 512:SEG], INF)
                wmin = _chain_full(nc, work, wb2, "w")
                for k in range(HC):
                    for hc in range(HC):
                        nc.tensor.transpose(
                            pt[:, k, 128 * hc:128 * (hc + 1)],
                            wmin[:, SEG * hc + 1 + 128 * k:
                                 SEG * hc + 1 + 128 * (k + 1)],
                            ident)
                nc.gpsimd.memset(hbuf[:, :, 0:PAD_L], INF)
                nc.gpsimd.memset(hbuf[:, :, PAD_L + 512:SEG], INF)
                pts.append(pt)
                hbufs.append(hbuf)

            first = True
            for b in range(B_LOC):
                hb2 = hbufs[b].rearrange("p a b -> p (a b)")
                for kw in range(2):          # H-direction half-streams
                    nc.scalar.copy(
                        out=hbufs[b][:, 2 * kw:2 * kw + 2, PAD_L:PAD_L + 512],
                        in_=pts[b][:, 2 * kw:2 * kw + 2, :])
                    hmin = _chain_half(nc, work, hb2, HALF * kw, "h")
                    for kl in range(2):
                        nc.tensor.matmul(
                            acc[0:1, :], ones,
                            hmin[:, SEG * kl + 1:SEG * kl + 513],
                            start=first,
                            stop=(b == B_LOC - 1 and kw == 1 and kl == 1))
                        first = False

            total = consts.tile([1, 1], f32)
            nc.vector.reduce_sum(
                out=total, in_=acc[0:1, :], axis=mybir.AxisListType.X)
            nc.sync.dma_start(out=out_d[:, :], in_=total)

    nc.compile()
    _CACHE["nc"] = nc
    return nc


def run(x, trace=False):
    """x: [16,3,512,512] float32. Returns (loss_scalar, exec_time_ns)."""
    nc = _build()
    xb = np.ascontiguousarray(x).astype(ml_dtypes.bfloat16)
    in_maps = [
        {"x": np.ascontiguousarray(xb[i * B_LOC:(i + 1) * B_LOC])}
        for i in range(N_CORES)
    ]
    res = run_bass_kernel_spmd(
        nc, in_maps, core_ids=list(range(N_CORES)), trace=trace)
    total = sum(float(r["out"][0, 0]) for r in res.results)
    loss = total / float(B * H * W) - 1.0
    return np.float32(loss), res.exec_time_ns


def kernel(x):
    loss, _ = run(x)
    return loss


# revision 15
# speedup vs baseline: 1.2001x; 1.2001x over previous
"""Trainium2 Bass kernel for nn_DarkCLoss: loss = -mean(|maxpool3d_{3,35,35}(1-x)|).

Math: with p=35 and -inf padding, the reference is
    loss = -mean(1 - minpool2d_35x35(min_c x)) = mean(minpool) - 1
so we compute the 2D sliding-window min (window 35, stride 1, +inf pads)
of the channel-min, sum it, and finish on the host.

Sharding: pure data-parallel, 2 images per core across 8 cores; each core
returns its partial sum of the pooled map; host combines (the scalar
all-reduce from the sharding hint, done on host).

Device algorithm per image (all pooling exact in bf16; inputs shipped as
bf16 — the pooled term is ~2.7e-4 of the loss, so bf16 rounding of the
input perturbs the result by ~1e-6 relative):
  - rows are laid out h = 128*hc + p; the 4 row-blocks become +inf-padded
    548-wide segments side by side in the free dim.
  - work is split into half-image streams (2 segments each) so DMA,
    VectorE, ScalarE and PE pipelines of the two images interleave.
  - channel min: DVE tensor_tensor on FLAT [128, 1096] views (flat 2D
    APs keep the DVE in its 2x bf16 perf mode; segment-crossing reads
    only pollute positions no valid output depends on, because every
    valid 35-window's dependency cone stays inside one padded segment).
  - sliding-min-35 = log2 doubling chain of shifted flat tensor_tensor
    mins (shifts 1,2,4,8,16,3).  Odd shifts are made 4-byte aligned by
    materializing the shifted operand with a ScalarE copy, so every DVE
    op stays in 2x mode.
  - PE transposes [128,128] blocks into PSUM; ScalarE copies PSUM into
    the padded H buffer; same chain along H on transposed halves.
  - PE ones-matmul accumulates the partition sums of the pooled map into
    one PSUM bank across both images; one DVE reduce drains it to a
    scalar that is DMA'd out.
"""

import numpy as np
import ml_dtypes

import concourse.bacc as bacc
import concourse.tile as tile
import concourse.mybir as mybir
from concourse.alu_op_type import AluOpType
from concourse.bass_utils import run_bass_kernel_spmd
from concourse.masks import make_identity

N_CORES = 8
B, C, H, W = 16, 3, 512, 512
B_LOC = B // N_CORES          # images per core
K = 35                        # pool window
PAD_L = 18                    # left pad (data starts 4B-aligned)
SEG = 548                     # padded segment width (= 18 + 512 + 18)
HC = 4                        # 512 rows = 4 blocks of 128 partitions
HALF = 2 * SEG                # 1096: one half-image stream (2 segments)
INF = float("inf")

# chain op widths for a 2-segment stream: SEG + per-segment need
W_M2, W_D2, W_D4, W_D8, W_D16, W_FIN = 1094, 1092, 1088, 1080, 1064, 1062

_CACHE = {}


def _chain_half(nc, pool, buf2, base, tag):
    """Sliding-min-35 over two padded segments buf2[:, base:base+HALF].

    buf2: flat [128, >=base+HALF] bf16 AP with inf pads.  Returns a flat
    [128, HALF] tile whose columns SEG*s + (1..512), s in {0,1}, hold
    the valid window mins.  All DVE ops are flat 2D and 4B-aligned (odd
    shifts via ScalarE shadow copies) -> 2x bf16 mode.
    """
    bf16 = mybir.dt.bfloat16
    mn = AluOpType.min

    def tl(name):
        return pool.tile([128, HALF], bf16, name=name, tag=name, bufs=3)

    sh1 = tl(f"sh1{tag}")
    nc.scalar.copy(out=sh1[:, 0:W_M2], in_=buf2[:, base + 1:base + 1 + W_M2])
    m2 = tl(f"cha{tag}")
    nc.vector.tensor_tensor(
        out=m2[:, 0:W_M2], in0=buf2[:, base:base + W_M2],
        in1=sh1[:, 0:W_M2], op=mn)
    m4 = tl(f"chb{tag}")
    nc.vector.tensor_tensor(
        out=m4[:, 0:W_D2], in0=m2[:, 0:W_D2], in1=m2[:, 2:W_D2 + 2], op=mn)
    m8 = tl(f"chc{tag}")
    nc.vector.tensor_tensor(
        out=m8[:, 0:W_D4], in0=m4[:, 0:W_D4], in1=m4[:, 4:W_D4 + 4], op=mn)
    m16 = tl(f"chd{tag}")
    nc.vector.tensor_tensor(
        out=m16[:, 0:W_D8], in0=m8[:, 0:W_D8], in1=m8[:, 8:W_D8 + 8], op=mn)
    m32 = tl(f"che{tag}")
    nc.vector.tensor_tensor(
        out=m32[:, 0:W_D16], in0=m16[:, 0:W_D16], in1=m16[:, 16:W_D16 + 16],
        op=mn)
    sh3 = tl(f"sh3{tag}")
    nc.scalar.copy(out=sh3[:, 0:W_FIN], in_=m32[:, 3:3 + W_FIN])
    out = tl(f"out{tag}")
    nc.vector.tensor_tensor(
        out=out[:, 0:W_FIN], in0=m32[:, 0:W_FIN], in1=sh3[:, 0:W_FIN], op=mn)
    return out


def _build():
    if "nc" in _CACHE:
        return _CACHE["nc"]
    bf16 = mybir.dt.bfloat16
    f32 = mybir.dt.float32
    mn = AluOpType.min

    nc = bacc.Bacc("TRN2", target_bir_lowering=False, debug=False)
    x = nc.dram_tensor("x", [B_LOC, C, H, W], bf16, kind="ExternalInput")
    out_d = nc.dram_tensor("out", [1, 1], f32, kind="ExternalOutput")

    with tile.TileContext(nc) as tc:
        with (
            tc.tile_pool(name="consts", bufs=1) as consts,
            tc.tile_pool(name="work", bufs=2) as work,
            tc.tile_pool(name="pswork", bufs=2, space="PSUM") as pswork,
            tc.tile_pool(name="psacc", bufs=1, space="PSUM") as psacc,
        ):
            ident = consts.tile([128, 128], bf16)
            make_identity(nc, ident)
            ones = consts.tile([128, 1], bf16)
            nc.vector.memset(ones, 1.0)
            acc = psacc.tile([1, 512], f32)

            pts, hbufs = [], []
            for b in range(B_LOC):
                pt = pswork.tile([128, HC, 512], bf16, name="pt")
                hbuf = work.tile([128, HC, SEG], bf16, name="hbuf", tag="hbuf")
                for hw in range(2):          # W-direction half-streams
                    ct = []
                    for c in range(C):
                        t = work.tile(
                            [128, 2, SEG], bf16, name=f"c{c}", tag=f"c{c}",
                            bufs=3)
                        src = x[b, c, 256 * hw:256 * (hw + 1)].rearrange(
                            "(hc p) w -> p hc w", p=128)
                        eng = nc.sync if c % 2 == 0 else nc.scalar
                        eng.dma_start(out=t[:, :, PAD_L:PAD_L + 512], in_=src)
                        ct.append(t)
                    cf = [t.rearrange("p a b -> p (a b)") for t in ct]
                    t1 = work.tile(
                        [128, HALF], bf16, name="t1", tag="t1", bufs=3)
                    nc.vector.tensor_tensor(out=t1, in0=cf[0], in1=cf[1], op=mn)
                    wbuf = work.tile(
                        [128, 2, SEG], bf16, name="wbuf", tag="wbuf", bufs=3)
                    nc.vector.tensor_tensor(
                        out=wbuf.rearrange("p a b -> p (a b)"), in0=t1,
                        in1=cf[2], op=mn)
                    nc.gpsimd.memset(wbuf[:, :, 0:PAD_L], INF)
                    nc.gpsimd.memset(wbuf[:, :, PAD_L + 512:SEG], INF)
                    wmin = _chain_half(
                        nc, work, wbuf.rearrange("p a b -> p (a b)"), 0, "w")
                    # transpose this half's rows into all 4 w-chunk tiles
                    for k in range(HC):
                        for hl in range(2):
                            hc = 2 * hw + hl
                            nc.tensor.transpose(
                                pt[:, k, 128 * hc:128 * (hc + 1)],
                                wmin[:, SEG * hl + 1 + 128 * k:
                                     SEG * hl + 1 + 128 * (k + 1)],
                                ident)
                nc.gpsimd.memset(hbuf[:, :, 0:PAD_L], INF)
                nc.gpsimd.memset(hbuf[:, :, PAD_L + 512:SEG], INF)
                pts.append(pt)
                hbufs.append(hbuf)

            first = True
            for b in range(B_LOC):
                hb2 = hbufs[b].rearrange("p a b -> p (a b)")
                for kw in range(2):          # H-direction half-streams
                    nc.scalar.copy(
                        out=hbufs[b][:, 2 * kw:2 * kw + 2, PAD_L:PAD_L + 512],
                        in_=pts[b][:, 2 * kw:2 * kw + 2, :])
                    hmin = _chain_half(nc, work, hb2, HALF * kw, "h")
                    for kl in range(2):
                        nc.tensor.matmul(
                            acc[0:1, :], ones,
                            hmin[:, SEG * kl + 1:SEG * kl + 513],
                            start=first,
                            stop=(b == B_LOC - 1 and kw == 1 and kl == 1))
                        first = False

            total = consts.tile([1, 1], f32)
            nc.vector.reduce_sum(
                out=total, in_=acc[0:1, :], axis=mybir.AxisListType.X)
            nc.sync.dma_start(out=out_d[:, :], in_=total)

    nc.compile()
    _CACHE["nc"] = nc
    return nc


def run(x, trace=False):
    """x: [16,3,512,512] float32. Returns (loss_scalar, exec_time_ns)."""
    nc = _build()
    xb = np.ascontiguousarray(x).astype(ml_dtypes.bfloat16)
    in_maps = [
        {"x": np.ascontiguousarray(xb[i * B_LOC:(i + 1) * B_LOC])}
        for i in range(N_CORES)
    ]
    res = run_bass_kernel_spmd(
        nc, in_maps, core_ids=list(range(N_CORES)), trace=trace)
    total = sum(float(r["out"][0, 0]) for r in res.results)
    loss = total / float(B * H * W) - 1.0
    return np.float32(loss), res.exec_time_ns


def kernel(x):
    loss, _ = run(x)
    return loss
